# revision 1
# baseline (speedup 1.0000x reference)
"""GPT2 causal attention (B=2, T=2048, C=1024, H=16) on 8 TRN2 NeuronCores.

Sharding: core g = (batch b = g//4, head-group hg = g%4 of 4 heads).
Tensor-parallel over heads (column-split W_attn, row-split W_proj) x
data-parallel over batch. Each core computes a full [T, C] partial of the
output projection for its 4 heads; host sums the 4 partials per batch and
adds b_proj. No collectives.

Per-core kernel (bf16 matmuls, fp32 PSUM). The cost model charges a matmul
only for its moving-free (output column) size, so every stage is oriented
to minimize total streamed columns:
  - Q/K dc0 are accumulated c-outer across 8 PSUM groups so the PE starts
    as soon as the first xT chunk lands (instead of waiting for all of xT).
  - Scores are computed transposed, S^T[tk, tq] = kT_i^T @ qT_j, with the
    causally-dead left part of diagonal-band tiles trimmed from the stream.
    exp() on Act (no max-subtraction; scores ~ N(0,1)), diagonal 128x128
    chunk masked with a host tri mask.
  - A*V uses the exp'd score chunk as the STATIONARY operand and V (with a
    ones column appended) as the moving operand: yps[tq, 65] += E_i^T @
    V_aug accumulates in PSUM over tk tiles; col 64 is the softmax
    denominator. This is 65 streamed columns per (tk-tile, q-chunk) vs 512
    for the yT orientation.
  - Per finished q-chunk: DVE reciprocal + per-partition scalar multiply
    normalizes into a [128 q, 128 (2 heads x 64 d)] SBUF tile, a PE
    transpose flips it into the [d, tq] layout the output projection needs,
    and (for the second head pair) the output projection for that 128-row
    band runs immediately and DMAs out fp16 partials -- so the projection
    and output DMA fully overlap attention instead of trailing it.
  - Remaining QKV work (V tiles 4..15, Q/K dc1) is emitted as self-contained
    filler groups inside the first head-pair's attention, filling the PE
    while Act (exp) is the per-tile critical path.
"""

import numpy as np
import ml_dtypes

BF16 = ml_dtypes.bfloat16

B, T, C, H, D = 2, 2048, 1024, 16, 64
HL = 4          # heads per core
DL = HL * D     # 256 local head dims
N_CORES = 8
NT = T // 128   # 16 tk tiles
NJ = T // 512   # 4 tq groups per head pair
SCALE = 1.0 / np.sqrt(D)

_CACHE = {}


def _build_program():
    import concourse.tile as tile
    from concourse import bacc
    import concourse.mybir as mybir

    f32 = mybir.dt.float32
    f16 = mybir.dt.float16
    bf16 = mybir.dt.bfloat16
    f8 = mybir.dt.float8e4
    DR = mybir.MatmulPerfMode.DoubleRow
    Exp = mybir.ActivationFunctionType.Exp

    nc = bacc.Bacc("TRN2", target_bir_lowering=False, debug=False)

    # ---- DRAM I/O (host pre-sharded and pre-packed to SBUF layout) ----
    xT_d = nc.dram_tensor("xTp", [128, 8 * T], bf16, kind="ExternalInput").ap()
    wq_d = nc.dram_tensor("wqp", [128, 8 * DL], bf16, kind="ExternalInput").ap()
    wk_d = nc.dram_tensor("wkp", [128, 8 * DL], bf16, kind="ExternalInput").ap()
    wv_d = nc.dram_tensor("wvp", [128, 8 * DL], bf16, kind="ExternalInput").ap()
    wp_d = nc.dram_tensor("wpp", [128, 2 * C], bf16, kind="ExternalInput").ap()
    bq_d = nc.dram_tensor("bq", [128, 2], f32, kind="ExternalInput").ap()
    bk_d = nc.dram_tensor("bk", [128, 2], f32, kind="ExternalInput").ap()
    bvr_d = nc.dram_tensor("bvr", [128, DL], f32, kind="ExternalInput").ap()
    tri_d = nc.dram_tensor("tri", [128, 128], bf16, kind="ExternalInput").ap()
    eye_d = nc.dram_tensor("eye", [128, 128], bf16, kind="ExternalInput").ap()
    out_d = nc.dram_tensor("out", [T, C], f16, kind="ExternalOutput").ap()
    LVL = int(__import__("os").environ.get("KLVL", "9"))
    KAV = int(__import__("os").environ.get("KAV", "9"))
    DBG = bool(__import__("os").environ.get("KDBG"))
    if DBG:
        qT_dump = nc.dram_tensor("qTd", [128, 2 * T], bf16, kind="ExternalOutput").ap()
        kT_dump = nc.dram_tensor("kTd", [128, 2 * T], bf16, kind="ExternalOutput").ap()
        yT_dump = nc.dram_tensor("yTd", [128, 2 * T], bf16, kind="ExternalOutput").ap()
        V_dump = nc.dram_tensor("Vd", [128, NT * HL * 65], bf16, kind="ExternalOutput").ap()
        yps_dump = nc.dram_tensor("ypsd", [128, 16 * 132], f32, kind="ExternalOutput").ap()
        rc_dump = nc.dram_tensor("rcd", [128, 16 * 2], f32, kind="ExternalOutput").ap()

    with tile.TileContext(nc) as tc:
        with (
            tc.tile_pool(name="const", bufs=1) as cpool,
            tc.tile_pool(name="exp", bufs=8) as epool,
            tc.tile_pool(name="small", bufs=8) as spool,
            tc.tile_pool(name="ostage", bufs=6) as opool,
            tc.tile_pool(name="pssc", bufs=2, space="PSUM") as pool_sc,
            tc.tile_pool(name="psy", bufs=2, space="PSUM") as pool_yps,
            tc.tile_pool(name="pspt", bufs=2, space="PSUM") as pool_pt,
        ):
            # ---- persistent SBUF ----
            xT = cpool.tile([128, 8 * T], bf16, tag="xT")       # c-chunk c at [:, c*T:]
            wq = cpool.tile([128, 8 * DL], bf16, tag="wq")
            wk = cpool.tile([128, 8 * DL], bf16, tag="wk")
            wv = cpool.tile([128, 8 * DL], bf16, tag="wv")
            wp = cpool.tile([128, 2 * C], bf16, tag="wp")       # d-chunk dc at [:, dc*C:]
            bq = cpool.tile([128, 2], f32, tag="bq")
            bk = cpool.tile([128, 2], f32, tag="bk")
            bvr = cpool.tile([128, DL], f32, tag="bvr")
            tri = cpool.tile([128, 128], bf16, tag="tri")
            eye = cpool.tile([128, 128], bf16, tag="eye")
            qT = cpool.tile([128, 2 * T], bf16, tag="qT")       # head h: [64*(h%2):, (h//2)*T + t]
            kT = cpool.tile([128, 2 * T], bf16, tag="kT")
            yT = cpool.tile([128, 2 * T], bf16, tag="yT")       # pair hp at [:, hp*T + t]
            # V_aug bf16, head-major: slot (h,tt) at [:, h*NT*65 + tt*65 : +65],
            # col 64 = ones (so A*V also yields the softmax row-sums).
            V = cpool.tile([128, HL * NT * 65], bf16, tag="V")

            Vv = V[:, :].rearrange("p (h t e) -> p h t e", h=HL, t=NT)
            nc.vector.memset(Vv[:, :, :, 64:65], 1.0)

            # ---- load inputs. wq/wk are packed dc-major [dc*1024 + c*128] so
            # the dc0 halves (all Phase-1 needs) land before the xT chunks;
            # transfers serialize on the DMA engine so order = arrival order.
            nc.sync.dma_start(out=wq[:, 0:512], in_=wq_d[:, 0:512])
            nc.sync.dma_start(out=xT[:, 0:512], in_=xT_d[:, 0:512])
            nc.sync.dma_start(out=wq[:, 512:1024], in_=wq_d[:, 512:1024])
            nc.sync.dma_start(out=xT[:, 512:1024], in_=xT_d[:, 512:1024])
            nc.sync.dma_start(out=wk[:, 0:1024], in_=wk_d[:, 0:1024])
            nc.sync.dma_start(out=xT[:, 1024:T], in_=xT_d[:, 1024:T])
            for c in range(1, 5):
                nc.sync.dma_start(out=xT[:, c * T:(c + 1) * T],
                                  in_=xT_d[:, c * T:(c + 1) * T])
            nc.sync.dma_start(out=wv[:, :], in_=wv_d[:, :])
            for c in range(5, 8):
                nc.sync.dma_start(out=xT[:, c * T:(c + 1) * T],
                                  in_=xT_d[:, c * T:(c + 1) * T])
            nc.sync.dma_start(out=bq[:, :], in_=bq_d[:, :])
            nc.sync.dma_start(out=bk[:, :], in_=bk_d[:, :])
            nc.sync.dma_start(out=bvr[:, :], in_=bvr_d[:, :])
            nc.sync.dma_start(out=tri[:, :], in_=tri_d[:, :])
            nc.sync.dma_start(out=eye[:, :], in_=eye_d[:, :])
            nc.sync.dma_start(out=wq[:, 1024:2048], in_=wq_d[:, 1024:2048])
            nc.sync.dma_start(out=wk[:, 1024:2048], in_=wk_d[:, 1024:2048])
            nc.sync.dma_start(out=wp[:, :], in_=wp_d[:, :])

            # ---- Phase 1: Q/K dc0, c-outer (8 live accumulation groups) ----
            qacc = [pool_sc.tile([128, 1024], f32, tag="sc", name=f"qacc{m}")
                    for m in range(2)]
            kacc = ([pool_yps.tile([128, 512], f32, tag="yps", name=f"kacc{m}")
                     for m in range(2)]
                    + [pool_pt.tile([128, 512], f32, tag="pt", name=f"kacc{m+2}")
                       for m in range(2)])
            for c in range(8):
                for tsp in range(2):  # ts pairs, Q then K, so the first
                    for ts in (2 * tsp, 2 * tsp + 1):  # mms match DMA arrival
                        nc.tensor.matmul(
                            qacc[ts // 2][:, (ts % 2) * 512:(ts % 2 + 1) * 512],
                            wq[:, c * 128:(c + 1) * 128],
                            xT[:, c * T + ts * 512: c * T + (ts + 1) * 512],
                            start=(c == 0), stop=(c == 7),
                        )
                    for ts in (2 * tsp, 2 * tsp + 1):
                        nc.tensor.matmul(
                            kacc[ts][:, :],
                            wk[:, c * 128:(c + 1) * 128],
                            xT[:, c * T + ts * 512: c * T + (ts + 1) * 512],
                            start=(c == 0), stop=(c == 7),
                        )
            # evicts staged so the PE unblocks asap: kacc2/3 (gpsimd) free the
            # pool_pt bufs the V-groups need; qacc ts0 / kacc0 (DVE) feed the
            # first attention block; the rest follow behind the V-groups.
            nc.vector.tensor_scalar_add(kT[:, 2 * 512:3 * 512], kacc[2][:, :],
                                        bk[:, 0:1])
            nc.vector.tensor_scalar_add(kT[:, 3 * 512:4 * 512], kacc[3][:, :],
                                        bk[:, 0:1])
            nc.vector.tensor_scalar_add(qT[:, 0:512], qacc[0][:, 0:512],
                                        bq[:, 0:1])
            nc.vector.tensor_scalar_add(kT[:, 0:512], kacc[0][:, :], bk[:, 0:1])

            # ---- self-contained QKV filler groups ----
            def emit_v_group(tt):
                ps = pool_pt.tile([128, DL], f32, tag="pt", name=f"vps{tt}")
                for c in range(8):
                    nc.tensor.matmul(
                        ps[:, :],
                        xT[:, c * T + tt * 128: c * T + (tt + 1) * 128],
                        wv[:, c * DL:(c + 1) * DL],
                        start=(c == 0), stop=(c == 7),
                    )
                nc.vector.tensor_add(
                    Vv[:, :, tt, 0:64],
                    ps[:, :].rearrange("p (h e) -> p h e", h=HL),
                    bvr[:, :].rearrange("p (h e) -> p h e", h=HL),
                )

            def emit_qk_group(w_sb, b_sb, dst, ts):
                ps = pool_pt.tile([128, 512], f32, tag="pt", name=f"qk1_{ts}")
                for c in range(8):
                    nc.tensor.matmul(
                        ps[:, :],
                        w_sb[:, 1024 + c * 128: 1024 + (c + 1) * 128],
                        xT[:, c * T + ts * 512: c * T + (ts + 1) * 512],
                        start=(c == 0), stop=(c == 7),
                    )
                nc.vector.tensor_scalar_add(
                    dst[:, T + ts * 512: T + (ts + 1) * 512], ps[:, :], b_sb[:, 1:2],
                )

            for tt in range(4):  # V tiles needed by (hp0, j0)
                if LVL >= 3:
                    emit_v_group(tt)
            for ts in range(1, 4):
                nc.vector.tensor_scalar_add(
                    qT[:, ts * 512:(ts + 1) * 512],
                    qacc[ts // 2][:, (ts % 2) * 512:(ts % 2 + 1) * 512],
                    bq[:, 0:1],
                )
            nc.vector.tensor_scalar_add(kT[:, 512:1024], kacc[1][:, :],
                                        bk[:, 0:1])

            fillers = [(lambda tt=tt: emit_v_group(tt)) for tt in range(4, NT)]
            fillers += [(lambda ts=ts: emit_qk_group(wq, bq, qT, ts)) for ts in range(4)]
            fillers += [(lambda ts=ts: emit_qk_group(wk, bk, kT, ts)) for ts in range(4)]
            fillers.reverse()  # pop() from the front

            # ---- attention ----
            def emit_proj(tt):
                # out[tt band, :] = sum_dc yT[dc, tt]^T @ wp[dc]; fp16 partial out
                for cc in range(2):
                    pp = pool_pt.tile([128, 512], f32, tag="pt", name=f"pp{tt}_{cc}")
                    for dc in range(2):
                        nc.tensor.matmul(
                            pp[:, :],
                            yT[:, dc * T + tt * 128: dc * T + (tt + 1) * 128],
                            wp[:, dc * C + cc * 512: dc * C + (cc + 1) * 512],
                            start=(dc == 0), stop=(dc == 1),
                        )
                    ot = opool.tile([128, 512], f16, tag="ot", name=f"ot{tt}_{cc}")
                    if (tt + cc) % 2 == 0:
                        nc.scalar.copy(ot[:, :], pp[:, :])
                    else:
                        nc.vector.tensor_copy(ot[:, :], pp[:, :])
                    nc.sync.dma_start(
                        out=out_d[tt * 128:(tt + 1) * 128, cc * 512:(cc + 1) * 512],
                        in_=ot[:, :],
                    )

            def emit_completion(hp, j, q4, ytiles):
                # q-chunk jj = 4j+q4 finished accumulating: normalize both
                # heads' [128 q, 64] + denominators (col 64 of each slot),
                # transpose to yT layout, then (hp==1) project that band.
                jj = 4 * j + q4
                yt = ytiles[q4 // 2]
                base = 132 * (q4 % 2)
                dn = yt[:, :].rearrange("p (s e) -> p s e", s=4)[
                    :, 2 * (q4 % 2):2 * (q4 % 2) + 2, 64]
                rc = spool.tile([128, 2], f32, tag="rc", name=f"rc{hp}_{jj}")
                nc.vector.reciprocal(rc[:, :], dn)
                if DBG and hp == 0:
                    ydbg = spool.tile([128, 132], f32, tag="ydbg",
                                      name=f"ydbg{jj}")
                    nc.vector.tensor_copy(ydbg[:, :], yt[:, base:base + 132])
                    nc.sync.dma_start(out=yps_dump[:, jj * 132:(jj + 1) * 132],
                                      in_=ydbg[:, :])
                    nc.sync.dma_start(out=rc_dump[:, jj * 2:(jj + 1) * 2],
                                      in_=rc[:, :])
                yp = spool.tile([128, 128], bf16, tag="yp", name=f"yp{hp}_{jj}")
                for half in range(2):
                    nc.vector.tensor_scalar_mul(
                        yp[:, half * 64:(half + 1) * 64],
                        yt[:, base + half * 66: base + half * 66 + 64],
                        rc[:, half:half + 1],
                    )
                tp = pool_pt.tile([128, 128], bf16, tag="pt", name=f"tp{hp}_{jj}")
                nc.tensor.transpose(tp[:, :], yp[:, :], eye[:, :])
                nc.vector.tensor_copy(yT[:, hp * T + jj * 128: hp * T + (jj + 1) * 128],
                                      tp[:, :])
                if hp == 1 and LVL >= 6:
                    emit_proj(jj)

            def attn_block(hp, j, do_fill):
                fb = hp * T
                ni = 4 * j + 4
                # yps slot (q4, half) = 2*q4+half: slots 0-3 in ya, 4-7 in yb;
                # 66 cols each (65 used: col 64 = softmax denominator).
                ytiles = [
                    pool_yps.tile([128, 264], f32, tag="yps", name=f"y{hp}_{j}_{m}")
                    for m in range(2)
                ]
                ets = [None] * ni

                def emit_score(i):
                    d0 = max(128 * (i - 4 * j), 0)
                    sc = pool_sc.tile([128, 1024], f32, tag="sc",
                                      name=f"sc{hp}_{j}_{i}")
                    for half in range(2):
                        po = 64 * half
                        nc.tensor.matmul(
                            sc[:, half * 512 + d0:(half + 1) * 512],
                            kT[po:po + 64, fb + i * 128: fb + (i + 1) * 128],
                            qT[po:po + 64, fb + j * 512 + d0: fb + (j + 1) * 512],
                            start=True, stop=True,
                        )
                    et = epool.tile([128, 1024], bf16, tag="exp",
                                    name=f"et{hp}_{j}_{i}")
                    et2 = et[:, :].rearrange("p (g q) -> p g q", g=2)
                    sc2 = sc[:, :].rearrange("p (g q) -> p g q", g=2)
                    nc.scalar.activation(
                        et2[:, :, d0:512], sc2[:, :, d0:512], Exp,
                        scale=float(SCALE),
                    )
                    if i >= 4 * j:  # diagonal chunk: causal mask (post-exp)
                        for half in range(2):
                            sl = slice(half * 512 + d0, half * 512 + d0 + 128)
                            nc.vector.tensor_mul(et[:, sl], et[:, sl], tri[:, :])
                    ets[i] = et

                def emit_av(i):
                    # PSUM start_tensor_calc marks the whole 2KB bank pending-
                    # zero, so: ONE start per yps bank (its first matmul); the
                    # other slots' first writes land on pending-zero bytes and
                    # overwrite; ONE stop on the bank's last matmul.
                    et = ets[i]
                    for half in range(2):
                        h = 2 * hp + half
                        for q4 in range(4):
                            if 4 * j + q4 < i:
                                continue
                            s = 2 * q4 + half
                            yt = ytiles[s // 4]
                            off = (s % 4) * 66
                            bank_start = (i == 0 and half == 0 and q4 % 2 == 0)
                            bank_stop = (half == 1 and q4 % 2 == 1
                                         and i == 4 * j + q4)
                            nc.tensor.matmul(
                                yt[:, off:off + 65],
                                et[:, half * 512 + q4 * 128: half * 512 + (q4 + 1) * 128],
                                Vv[:, h, i, :],
                                start=bank_start, stop=bank_stop,
                                skip_group_check=True,
                            )

                # 1-deep software pipeline: score(i+1) issues before av(i) so
                # the PE never waits on Act's exp(i).
                for i in range(ni + 1):
                    if i < ni:
                        emit_score(i)
                    if i >= 1:
                        if KAV >= 1:
                            emit_av(i - 1)
                        if KAV >= 2 and (i - 1) >= 4 * j:
                            emit_completion(hp, j, (i - 1) - 4 * j, ytiles)
                        if do_fill and fillers and (i % 2 == 0):
                            fillers.pop()()

            if LVL >= 4:
                if LVL == 4:  # bisect mode: no interleaved fillers
                    while fillers:
                        fillers.pop()()
                for j in range(NJ):
                    attn_block(0, j, do_fill=(LVL > 4))
                while fillers:  # leftovers before hp1 needs qT/kT dc1
                    fillers.pop()()
                if LVL >= 5:
                    for j in range(NJ):
                        attn_block(1, j, do_fill=False)

            if DBG:
                nc.sync.dma_start(out=qT_dump[:, :], in_=qT[:, :])
                nc.sync.dma_start(out=kT_dump[:, :], in_=kT[:, :])
                nc.sync.dma_start(out=yT_dump[:, :], in_=yT[:, :])
                nc.sync.dma_start(out=V_dump[:, :], in_=V[:, :])

    nc.compile()
    return nc


def get_program():
    if "nc" not in _CACHE:
        _CACHE["nc"] = _build_program()
    return _CACHE["nc"]


def _pack_cmajor(a):
    """[C_rows, N] -> [128, (C_rows/128)*N] with chunk c at [:, c*N:(c+1)*N]."""
    rows, n = a.shape
    return np.ascontiguousarray(
        a.reshape(rows // 128, 128, n).transpose(1, 0, 2).reshape(128, -1))


def make_in_maps(x, W_attn, b_attn, W_proj):
    """Host-side sharding: per-core input dict."""
    x = np.asarray(x, np.float32)
    W_attn = np.asarray(W_attn, np.float32)
    b_attn = np.asarray(b_attn, np.float32)
    W_proj = np.asarray(W_proj, np.float32)

    tk = np.arange(128)[:, None]
    tq = np.arange(128)[None, :]
    tri = (tq >= tk).astype(BF16)
    eye = np.eye(128, dtype=BF16)

    xT_b = [_pack_cmajor(x[b].T.astype(BF16)) for b in range(B)]

    in_maps = []
    def _pack_dcmajor(a):
        # [1024, 256] -> [128, dc*1024 + c*128]
        return np.concatenate(
            [_pack_cmajor(a[:, 0:128]), _pack_cmajor(a[:, 128:256])], axis=1)

    for g in range(N_CORES):
        b, hg = divmod(g, 4)
        cs = slice(hg * DL, (hg + 1) * DL)
        wq = _pack_dcmajor(W_attn[:, 0 * C:1 * C][:, cs].astype(BF16))
        wk = _pack_dcmajor(W_attn[:, 1 * C:2 * C][:, cs].astype(BF16))
        wv = _pack_cmajor(W_attn[:, 2 * C:3 * C][:, cs].astype(BF16))
        wp = _pack_cmajor(W_proj[cs, :].astype(BF16))
        bq = np.ascontiguousarray(b_attn[0 * C:1 * C][cs].reshape(2, 128).T)
        bk = np.ascontiguousarray(b_attn[1 * C:2 * C][cs].reshape(2, 128).T)
        bvr = np.ascontiguousarray(np.tile(b_attn[2 * C:3 * C][cs][None, :], (128, 1)))
        in_maps.append({
            "xTp": xT_b[b],
            "wqp": wq, "wkp": wk, "wvp": wv, "wpp": wp,
            "bq": bq.astype(np.float32), "bk": bk.astype(np.float32),
            "bvr": bvr.astype(np.float32),
            "tri": tri, "eye": eye,
        })
    return in_maps


def assemble_output(results, b_proj):
    """results: per-core dicts with 'out' [T, C] fp16 partials."""
    b_proj = np.asarray(b_proj, np.float32)
    out = np.zeros((B, T, C), np.float32)
    for g in range(N_CORES):
        out[g // 4] += np.asarray(results[g]["out"], np.float32)
    out += b_proj[None, None, :]
    return out


def kernel(x, W_attn, b_attn, W_proj, b_proj):
    from concourse.bass_utils import run_bass_kernel_spmd

    nc = get_program()
    in_maps = make_in_maps(x, W_attn, b_attn, W_proj)
    res = run_bass_kernel_spmd(nc, in_maps, list(range(N_CORES)))
    return assemble_output(res.results, b_proj)

